# revision 4
# baseline (speedup 1.0000x reference)
"""Expert-parallel MoE MLP kernel for TRN2 (8 NeuronCores).

Reference computation (all experts, dense routing):
    hidden = einsum("bnd,edh->benh", x, w1); hidden = gelu(hidden)
    out    = einsum("benh,ehd->bnde", hidden, w2)        # [b, n, d4, e]

Sharding: expert-parallel, 2 experts per core (16 experts / 8 cores); x is
replicated. Each core computes, for its experts e:
    hT[e] = gelu(W1[e].T @ X.T)        # [h, tok] layout, h on partitions
    outT[e] = W2[e].T @ hT[e]          # [d4, tok] layout
which keeps the contraction dim on SBUF partitions for both matmuls with no
on-device transposes: W1 (d,h) / W2 (h,d4) load in natural layout as lhsT, and
X.T is prepared once on the host.

All operands are bf16 (PSUM accumulation stays fp32): same PE throughput as
fp32r (1 row/cycle at N=512) but the compiler's automatic fast-weight-load
engages for 16-bit weights, hiding LDWEIGHTS under the previous matmul's
streaming, and DMA traffic halves (x 8MB, w 1.25MB, out 4MB per core).
The [e, d4, tok] bf16 device layout is cast and re-interleaved to the
[b, n, d4, e] fp32 output on the host.
"""

import sys

import numpy as np

for _p in ("/opt/trn_rl_repo", "/root/.axon_site/_ro/trn_rl_repo"):
    if _p not in sys.path:
        sys.path.append(_p)

import ml_dtypes

import concourse.bacc as bacc
import concourse.mybir as mybir
import concourse.tile as tile
from concourse.bass_utils import run_bass_kernel_spmd

F32 = mybir.dt.float32
BF16 = mybir.dt.bfloat16
NP_BF16 = ml_dtypes.bfloat16

N_CORES = 8
E = 16                 # total experts
E_LOC = E // N_CORES   # experts per core
D = 512                # model dim (contraction of mm1)
H = 512                # hidden dim (contraction of mm2)
D4 = 128               # output dim per expert
NTOK = 4 * 2048        # tokens
TT = 512               # token tile (matmul moving free dim)
P = 128


def _build_program():
    nc = bacc.Bacc("TRN2", target_bir_lowering=False, debug=False)
    xT = nc.declare_dram_parameter("xT", [D, NTOK], BF16, isOutput=False)
    w1 = nc.declare_dram_parameter("w1", [E_LOC, D, H], BF16, isOutput=False)
    w2 = nc.declare_dram_parameter("w2", [E_LOC, H, D4], BF16, isOutput=False)
    outT = nc.declare_dram_parameter("outT", [E_LOC, D4, NTOK], BF16, isOutput=True)

    gelu = mybir.ActivationFunctionType.Gelu
    n_dt = D // P   # 4 k-tiles of mm1
    n_ht = H // P   # 4 k-tiles of mm2

    n_t = NTOK // TT

    with tile.TileContext(nc) as tc:
        with (
            tc.tile_pool(name="wpool", bufs=1) as wpool,
            tc.tile_pool(name="xpool", bufs=4) as xpool,
            tc.tile_pool(name="hpool", bufs=2) as hpool,
            tc.tile_pool(name="opool", bufs=4) as opool,
            tc.tile_pool(name="ps1p", bufs=4, space="PSUM") as ps1p,
            tc.tile_pool(name="ps2p", bufs=4, space="PSUM") as ps2p,
        ):
            # Weights resident in SBUF for the whole kernel, natural layout.
            w1_sb = wpool.tile([P, E_LOC, n_dt, H], BF16, name="w1_sb", tag="w1")
            w1_r = w1.rearrange("e (dt p) h -> p e dt h", p=P)
            w2_sb = wpool.tile([P, E_LOC, n_ht, D4], BF16, name="w2_sb", tag="w2")
            w2_r = w2.rearrange("e (ht p) d -> p e ht d", p=P)
            xT_r = xT.rearrange("(dt p) n -> p dt n", p=P)

            x_tiles = {}

            def load_x(t):
                tok = slice(t * TT, (t + 1) * TT)
                x_sb = xpool.tile([P, n_dt, TT], BF16, name="x_sb", tag="x")
                # gpsimd queue: x loads issue in parallel with w/out DMAs on sync
                nc.gpsimd.dma_start(x_sb, xT_r[:, :, tok])
                x_tiles[t] = x_sb

            # Startup: per-k-tile x0/w1[e0] slice pairs issue on two queues in
            # parallel so the first matmul chain starts as early as possible;
            # tile0/e0 consumes them in dt-outer order (see below) to match the
            # arrival order. w1[e1] follows, ahead of w2.
            tok0 = slice(0, TT)
            x0_sb = xpool.tile([P, n_dt, TT], BF16, name="x_sb", tag="x")
            for dt_i in range(n_dt):
                nc.gpsimd.dma_start(x0_sb[:, dt_i], xT_r[:, dt_i, tok0])
                nc.sync.dma_start(w1_sb[:, 0, dt_i], w1_r[:, 0, dt_i])
            x_tiles[0] = x0_sb
            for e in range(1, E_LOC):
                nc.sync.dma_start(w1_sb[:, e], w1_r[:, e])
            nc.sync.dma_start(w2_sb, w2_r)

            def mm1(e, x_sb, dt_outer):
                """One expert's mm1 + gelu for a token tile -> hT tile."""
                hT_sb = hpool.tile([P, n_ht, TT], BF16, name="hT_sb", tag="h")
                ps1s = [ps1p.tile([P, TT], F32, name="ps1", tag="ps1") for _ in range(n_ht)]
                order = (
                    [(dt_i, ht) for dt_i in range(n_dt) for ht in range(n_ht)]
                    if dt_outer
                    else [(dt_i, ht) for ht in range(n_ht) for dt_i in range(n_dt)]
                )
                for dt_i, ht in order:
                    nc.tensor.matmul(
                        ps1s[ht],
                        w1_sb[:, e, dt_i, ht * P : (ht + 1) * P],
                        x_sb[:, dt_i],
                        start=(dt_i == 0),
                        stop=(dt_i == n_dt - 1),
                    )
                    if dt_i == n_dt - 1:
                        nc.scalar.activation(hT_sb[:, ht, :], ps1s[ht], gelu)
                return hT_sb

            def mm2(e, hT_sb, tok, n_split=1):
                ntt = TT // n_split
                for s in range(n_split):
                    ts_ = slice(s * ntt, (s + 1) * ntt)
                    ps2 = ps2p.tile([P, ntt], F32, name="ps2", tag="ps2")
                    for ht in range(n_ht):
                        nc.tensor.matmul(
                            ps2,
                            w2_sb[:, e, ht, :],
                            hT_sb[:, ht, ts_],
                            start=(ht == 0),
                            stop=(ht == n_ht - 1),
                        )
                    o_sb = opool.tile([P, ntt], BF16, name="o_sb", tag="o")
                    nc.vector.tensor_copy(o_sb, ps2)
                    nc.sync.dma_start(
                        outT[e, :, tok.start + s * ntt : tok.start + (s + 1) * ntt],
                        o_sb,
                    )

            for t in range(n_t):
                tok = slice(t * TT, (t + 1) * TT)
                if t not in x_tiles:
                    load_x(t)
                x_sb = x_tiles.pop(t)
                hT_tiles = [mm1(e, x_sb, dt_outer=(t == 0 and e == 0)) for e in range(E_LOC)]
                last = t == n_t - 1
                for e in range(E_LOC):
                    # split the final output so the kernel's last DMA is small
                    mm2(e, hT_tiles[e], tok, n_split=2 if (last and e == E_LOC - 1) else 1)

    nc.finalize()
    return nc


_NC = None


def _get_program():
    global _NC
    if _NC is None:
        _NC = _build_program()
    return _NC


def _prep_inputs(x, w1, w2):
    xT = np.ascontiguousarray(x.reshape(NTOK, D).T).astype(NP_BF16)
    w1b = w1.astype(NP_BF16)
    w2b = w2.astype(NP_BF16)
    return [
        {
            "xT": xT,
            "w1": np.ascontiguousarray(w1b[c * E_LOC : (c + 1) * E_LOC]),
            "w2": np.ascontiguousarray(w2b[c * E_LOC : (c + 1) * E_LOC]),
        }
        for c in range(N_CORES)
    ]


def kernel(x: np.ndarray, w1: np.ndarray, w2: np.ndarray, **_) -> np.ndarray:
    """Full inputs in, full output out; expert-parallel across 8 NeuronCores."""
    nc = _get_program()
    in_maps = _prep_inputs(x, w1, w2)
    res = run_bass_kernel_spmd(nc, in_maps, list(range(N_CORES)))

    full = np.stack(
        [np.asarray(res.results[c]["outT"]) for c in range(N_CORES)], axis=0
    ).astype(np.float32)
    full = full.reshape(E, D4, NTOK)              # [e, d4, tok]
    out = full.transpose(2, 1, 0)                 # [tok, d4, e]
    return np.ascontiguousarray(out.reshape(4, 2048, D4, E), dtype=np.float32)


# revision 6
# speedup vs baseline: 1.0458x; 1.0458x over previous
"""Expert-parallel MoE MLP kernel for TRN2 (8 NeuronCores).

Reference computation (all experts, dense routing):
    hidden = einsum("bnd,edh->benh", x, w1); hidden = gelu(hidden)
    out    = einsum("benh,ehd->bnde", hidden, w2)        # [b, n, d4, e]

Sharding: expert-parallel, 2 experts per core (16 experts / 8 cores); x is
replicated. Each core computes, for its experts e:
    hT[e] = gelu(W1[e].T @ X.T)        # [h, tok] layout, h on partitions
    outT[e] = W2[e].T @ hT[e]          # [d4, tok] layout
which keeps the contraction dim on SBUF partitions for both matmuls with no
on-device transposes: W1 (d,h) / W2 (h,d4) load in natural layout as lhsT, and
X.T is prepared once on the host.

All operands are bf16 (PSUM accumulation stays fp32): same PE throughput as
fp32r (1 row/cycle at N=512) but the compiler's automatic fast-weight-load
engages for 16-bit weights, hiding LDWEIGHTS under the previous matmul's
streaming, and DMA traffic halves (x 8MB, w 1.25MB, out 4MB per core).
The [e, d4, tok] bf16 device layout is cast and re-interleaved to the
[b, n, d4, e] fp32 output on the host.
"""

import sys

import numpy as np

for _p in ("/opt/trn_rl_repo", "/root/.axon_site/_ro/trn_rl_repo"):
    if _p not in sys.path:
        sys.path.append(_p)

import ml_dtypes

import concourse.bacc as bacc
import concourse.mybir as mybir
import concourse.tile as tile
from concourse.bass_utils import run_bass_kernel_spmd

F32 = mybir.dt.float32
BF16 = mybir.dt.bfloat16
NP_BF16 = ml_dtypes.bfloat16

N_CORES = 8
E = 16                 # total experts
E_LOC = E // N_CORES   # experts per core
D = 512                # model dim (contraction of mm1)
H = 512                # hidden dim (contraction of mm2)
D4 = 128               # output dim per expert
NTOK = 4 * 2048        # tokens
TT = 512               # token tile (matmul moving free dim)
P = 128


def _build_program():
    nc = bacc.Bacc("TRN2", target_bir_lowering=False, debug=False)
    xT = nc.declare_dram_parameter("xT", [D, NTOK], BF16, isOutput=False)
    w1 = nc.declare_dram_parameter("w1", [E_LOC, D, H], BF16, isOutput=False)
    w2 = nc.declare_dram_parameter("w2", [E_LOC, H, D4], BF16, isOutput=False)
    outT = nc.declare_dram_parameter("outT", [E_LOC, D4, NTOK], BF16, isOutput=True)

    gelu = mybir.ActivationFunctionType.Gelu
    n_dt = D // P   # 4 k-tiles of mm1
    n_ht = H // P   # 4 k-tiles of mm2

    n_t = NTOK // TT

    with tile.TileContext(nc) as tc:
        with (
            tc.tile_pool(name="wpool", bufs=1) as wpool,
            tc.tile_pool(name="xpool", bufs=4) as xpool,
            tc.tile_pool(name="hpool", bufs=2) as hpool,
            tc.tile_pool(name="opool", bufs=4) as opool,
            tc.tile_pool(name="ps1p", bufs=4, space="PSUM") as ps1p,
            tc.tile_pool(name="ps2p", bufs=4, space="PSUM") as ps2p,
        ):
            # Weights resident in SBUF for the whole kernel, natural layout.
            w1_sb = wpool.tile([P, E_LOC, n_dt, H], BF16, name="w1_sb", tag="w1")
            w1_r = w1.rearrange("e (dt p) h -> p e dt h", p=P)
            w2_sb = wpool.tile([P, E_LOC, n_ht, D4], BF16, name="w2_sb", tag="w2")
            w2_r = w2.rearrange("e (ht p) d -> p e ht d", p=P)
            xT_r = xT.rearrange("(dt p) n -> p dt n", p=P)

            # PE warmup: dummy matmuls with no DMA dependency keep the PE busy
            # through the initial x0/w1 transfer window so the p-state is fully
            # ramped (2.4 GHz) when the first real chain starts. Sized to end
            # right as the startup DMAs complete (~11.5us). The warmup PSUM
            # reuses a ps2 pool slot before its first real use.
            junk = wpool.tile([P, TT], BF16, name="junk", tag="junk")
            nc.vector.memset(junk, 0.0)
            wu_ps = ps2p.tile([P, TT], F32, name="wu_ps", tag="ps2")
            for _ in range(10):
                nc.tensor.matmul(wu_ps, junk[:, :P], junk, start=True, stop=True)

            x_tiles = {}

            def load_x(t):
                tok = slice(t * TT, (t + 1) * TT)
                x_sb = xpool.tile([P, n_dt, TT], BF16, name="x_sb", tag="x")
                nc.sync.dma_start(x_sb, xT_r[:, :, tok])
                x_tiles[t] = x_sb

            # Startup DMAs: x0 first, then w1[e0] in ht-column blocks so chain
            # (e0, ht0) only needs x0 + 128KB of w1; w1[e1] lands during
            # mm1(e0); w2 after.
            tok0 = slice(0, TT)
            x0_sb = xpool.tile([P, n_dt, TT], BF16, name="x_sb", tag="x")
            nc.sync.dma_start(x0_sb, xT_r[:, :, tok0])
            for ht in range(n_ht):
                nc.sync.dma_start(
                    w1_sb[:, 0, :, ht * P : (ht + 1) * P],
                    w1_r[:, 0, :, ht * P : (ht + 1) * P],
                )
            x_tiles[0] = x0_sb
            for e in range(1, E_LOC):
                nc.sync.dma_start(w1_sb[:, e], w1_r[:, e])
            nc.sync.dma_start(w2_sb, w2_r)

            def mm1(e, x_sb):
                """One expert's mm1 + gelu for a token tile -> hT tile."""
                hT_sb = hpool.tile([P, n_ht, TT], BF16, name="hT_sb", tag="h")
                for ht in range(n_ht):
                    ps1 = ps1p.tile([P, TT], F32, name="ps1", tag="ps1")
                    for dt_i in range(n_dt):
                        nc.tensor.matmul(
                            ps1,
                            w1_sb[:, e, dt_i, ht * P : (ht + 1) * P],
                            x_sb[:, dt_i],
                            start=(dt_i == 0),
                            stop=(dt_i == n_dt - 1),
                        )
                    nc.scalar.activation(hT_sb[:, ht, :], ps1, gelu)
                return hT_sb

            def mm2(e, hT_sb, tok, n_split=1):
                ntt = TT // n_split
                for s in range(n_split):
                    ts_ = slice(s * ntt, (s + 1) * ntt)
                    ps2 = ps2p.tile([P, ntt], F32, name="ps2", tag="ps2")
                    for ht in range(n_ht):
                        nc.tensor.matmul(
                            ps2,
                            w2_sb[:, e, ht, :],
                            hT_sb[:, ht, ts_],
                            start=(ht == 0),
                            stop=(ht == n_ht - 1),
                        )
                    o_sb = opool.tile([P, ntt], BF16, name="o_sb", tag="o")
                    nc.vector.tensor_copy(o_sb, ps2)
                    nc.sync.dma_start(
                        outT[e, :, tok.start + s * ntt : tok.start + (s + 1) * ntt],
                        o_sb,
                    )

            for t in range(n_t):
                tok = slice(t * TT, (t + 1) * TT)
                if t not in x_tiles:
                    load_x(t)
                x_sb = x_tiles.pop(t)
                hT_tiles = [mm1(e, x_sb) for e in range(E_LOC)]
                last = t == n_t - 1
                for e in range(E_LOC):
                    # split the final output so the kernel's last DMA is small
                    mm2(e, hT_tiles[e], tok, n_split=2 if (last and e == E_LOC - 1) else 1)

    nc.finalize()
    return nc


_NC = None


def _get_program():
    global _NC
    if _NC is None:
        _NC = _build_program()
    return _NC


def _prep_inputs(x, w1, w2):
    xT = np.ascontiguousarray(x.reshape(NTOK, D).T).astype(NP_BF16)
    w1b = w1.astype(NP_BF16)
    w2b = w2.astype(NP_BF16)
    return [
        {
            "xT": xT,
            "w1": np.ascontiguousarray(w1b[c * E_LOC : (c + 1) * E_LOC]),
            "w2": np.ascontiguousarray(w2b[c * E_LOC : (c + 1) * E_LOC]),
        }
        for c in range(N_CORES)
    ]


def kernel(x: np.ndarray, w1: np.ndarray, w2: np.ndarray, **_) -> np.ndarray:
    """Full inputs in, full output out; expert-parallel across 8 NeuronCores."""
    nc = _get_program()
    in_maps = _prep_inputs(x, w1, w2)
    res = run_bass_kernel_spmd(nc, in_maps, list(range(N_CORES)))

    full = np.stack(
        [np.asarray(res.results[c]["outT"]) for c in range(N_CORES)], axis=0
    ).astype(np.float32)
    full = full.reshape(E, D4, NTOK)              # [e, d4, tok]
    out = full.transpose(2, 1, 0)                 # [tok, d4, e]
    return np.ascontiguousarray(out.reshape(4, 2048, D4, E), dtype=np.float32)


# revision 7
# speedup vs baseline: 1.0469x; 1.0010x over previous
"""Expert-parallel MoE MLP kernel for TRN2 (8 NeuronCores).

Reference computation (all experts, dense routing):
    hidden = einsum("bnd,edh->benh", x, w1); hidden = gelu(hidden)
    out    = einsum("benh,ehd->bnde", hidden, w2)        # [b, n, d4, e]

Sharding: expert-parallel, 2 experts per core (16 experts / 8 cores); x is
replicated. Each core computes, for its experts e:
    hT[e] = gelu(W1[e].T @ X.T)        # [h, tok] layout, h on partitions
    outT[e] = W2[e].T @ hT[e]          # [d4, tok] layout
which keeps the contraction dim on SBUF partitions for both matmuls with no
on-device transposes: W1 (d,h) / W2 (h,d4) load in natural layout as lhsT, and
X.T is prepared once on the host.

All operands are bf16 (PSUM accumulation stays fp32): same PE throughput as
fp32r (1 row/cycle at N=512) but the compiler's automatic fast-weight-load
engages for 16-bit weights, hiding LDWEIGHTS under the previous matmul's
streaming, and DMA traffic halves (x 8MB, w 1.25MB, out 4MB per core).
The [e, d4, tok] bf16 device layout is cast and re-interleaved to the
[b, n, d4, e] fp32 output on the host.
"""

import sys

import numpy as np

for _p in ("/opt/trn_rl_repo", "/root/.axon_site/_ro/trn_rl_repo"):
    if _p not in sys.path:
        sys.path.append(_p)

import ml_dtypes

import concourse.bacc as bacc
import concourse.mybir as mybir
import concourse.tile as tile
from concourse.bass_utils import run_bass_kernel_spmd

F32 = mybir.dt.float32
BF16 = mybir.dt.bfloat16
NP_BF16 = ml_dtypes.bfloat16

N_CORES = 8
E = 16                 # total experts
E_LOC = E // N_CORES   # experts per core
D = 512                # model dim (contraction of mm1)
H = 512                # hidden dim (contraction of mm2)
D4 = 128               # output dim per expert
NTOK = 4 * 2048        # tokens
TT = 512               # token tile (matmul moving free dim)
P = 128


def _build_program():
    nc = bacc.Bacc("TRN2", target_bir_lowering=False, debug=False)
    xT = nc.declare_dram_parameter("xT", [D, NTOK], BF16, isOutput=False)
    w1 = nc.declare_dram_parameter("w1", [E_LOC, D, H], BF16, isOutput=False)
    w2 = nc.declare_dram_parameter("w2", [E_LOC, H, D4], BF16, isOutput=False)
    outT = nc.declare_dram_parameter("outT", [E_LOC, D4, NTOK], BF16, isOutput=True)

    gelu = mybir.ActivationFunctionType.Gelu
    n_dt = D // P   # 4 k-tiles of mm1
    n_ht = H // P   # 4 k-tiles of mm2

    n_t = NTOK // TT

    with tile.TileContext(nc) as tc:
        with (
            tc.tile_pool(name="wpool", bufs=1) as wpool,
            tc.tile_pool(name="xpool", bufs=4) as xpool,
            tc.tile_pool(name="hpool", bufs=2) as hpool,
            tc.tile_pool(name="opool", bufs=4) as opool,
            tc.tile_pool(name="ps1p", bufs=4, space="PSUM") as ps1p,
            tc.tile_pool(name="ps2p", bufs=4, space="PSUM") as ps2p,
        ):
            # Weights resident in SBUF for the whole kernel, natural layout.
            w1_sb = wpool.tile([P, E_LOC, n_dt, H], BF16, name="w1_sb", tag="w1")
            w1_r = w1.rearrange("e (dt p) h -> p e dt h", p=P)
            w2_sb = wpool.tile([P, E_LOC, n_ht, D4], BF16, name="w2_sb", tag="w2")
            w2_r = w2.rearrange("e (ht p) d -> p e ht d", p=P)
            xT_r = xT.rearrange("(dt p) n -> p dt n", p=P)

            # PE warmup: dummy matmuls with no DMA dependency keep the PE busy
            # through the initial x0/w1 transfer window so the p-state is fully
            # ramped (2.4 GHz) when the first real chain starts. Sized to end
            # right as the startup DMAs complete (~11.5us). The warmup PSUM
            # reuses a ps2 pool slot before its first real use.
            junk = wpool.tile([P, TT], BF16, name="junk", tag="junk")
            nc.vector.memset(junk, 0.0)
            wu_ps = ps2p.tile([P, TT], F32, name="wu_ps", tag="ps2")
            for _ in range(10):
                nc.tensor.matmul(wu_ps, junk[:, :P], junk, start=True, stop=True)

            x_tiles = {}

            def load_x(t):
                tok = slice(t * TT, (t + 1) * TT)
                x_sb = xpool.tile([P, n_dt, TT], BF16, name="x_sb", tag="x")
                nc.sync.dma_start(x_sb, xT_r[:, :, tok])
                x_tiles[t] = x_sb

            # Startup DMAs: x0 first, then w1[e0] in ht-column blocks so chain
            # (e0, ht0) only needs x0 + 128KB of w1; w1[e1] lands during
            # mm1(e0); w2 after.
            tok0 = slice(0, TT)
            x0_sb = xpool.tile([P, n_dt, TT], BF16, name="x_sb", tag="x")
            nc.sync.dma_start(x0_sb, xT_r[:, :, tok0])
            for ht in range(n_ht):
                nc.sync.dma_start(
                    w1_sb[:, 0, :, ht * P : (ht + 1) * P],
                    w1_r[:, 0, :, ht * P : (ht + 1) * P],
                )
            x_tiles[0] = x0_sb
            for e in range(1, E_LOC):
                nc.sync.dma_start(w1_sb[:, e], w1_r[:, e])
            nc.sync.dma_start(w2_sb, w2_r)

            def mm1(e, x_sb):
                """One expert's mm1 + gelu for a token tile -> hT tile."""
                hT_sb = hpool.tile([P, n_ht, TT], BF16, name="hT_sb", tag="h")
                for ht in range(n_ht):
                    ps1 = ps1p.tile([P, TT], F32, name="ps1", tag="ps1")
                    for dt_i in range(n_dt):
                        nc.tensor.matmul(
                            ps1,
                            w1_sb[:, e, dt_i, ht * P : (ht + 1) * P],
                            x_sb[:, dt_i],
                            start=(dt_i == 0),
                            stop=(dt_i == n_dt - 1),
                        )
                    nc.scalar.activation(hT_sb[:, ht, :], ps1, gelu)
                return hT_sb

            def mm2(e, hT_sb, tok, n_split=1):
                ntt = TT // n_split
                for s in range(n_split):
                    ts_ = slice(s * ntt, (s + 1) * ntt)
                    ps2 = ps2p.tile([P, ntt], F32, name="ps2", tag="ps2")
                    for ht in range(n_ht):
                        nc.tensor.matmul(
                            ps2,
                            w2_sb[:, e, ht, :],
                            hT_sb[:, ht, ts_],
                            start=(ht == 0),
                            stop=(ht == n_ht - 1),
                        )
                    o_sb = opool.tile([P, ntt], BF16, name="o_sb", tag="o")
                    nc.vector.tensor_copy(o_sb, ps2)
                    nc.sync.dma_start(
                        outT[e, :, tok.start + s * ntt : tok.start + (s + 1) * ntt],
                        o_sb,
                    )

            # Software-pipelined schedule: each tile's mm2 chains run AFTER the
            # next tile's mm1 has been interleaved, so mm2 never waits on the
            # gelu that produced its hT input (kills a ~216ns PE bubble/tile).
            # PE order: mm1(0,e0) mm1(0,e1) | mm2(0,e0) mm1(1,e0) mm2(0,e1)
            # mm1(1,e1) | mm2(1,e0) mm1(2,e0) ...
            load_x(1)
            x0 = x_tiles.pop(0)
            hT_cur = [mm1(e, x0) for e in range(E_LOC)]
            for t in range(n_t):
                tok = slice(t * TT, (t + 1) * TT)
                nxt = t + 1
                if nxt + 1 < n_t:
                    load_x(nxt + 1)
                x_nxt = x_tiles.pop(nxt) if nxt < n_t else None
                hT_nxt = [None] * E_LOC
                for e in range(E_LOC):
                    last_e = nxt >= n_t and e == E_LOC - 1
                    # split the final output so the kernel's last DMA is small
                    mm2(e, hT_cur[e], tok, n_split=2 if last_e else 1)
                    if nxt < n_t:
                        hT_nxt[e] = mm1(e, x_nxt)
                hT_cur = hT_nxt

    nc.finalize()
    return nc


_NC = None


def _get_program():
    global _NC
    if _NC is None:
        _NC = _build_program()
    return _NC


def _prep_inputs(x, w1, w2):
    xT = np.ascontiguousarray(x.reshape(NTOK, D).T).astype(NP_BF16)
    w1b = w1.astype(NP_BF16)
    w2b = w2.astype(NP_BF16)
    return [
        {
            "xT": xT,
            "w1": np.ascontiguousarray(w1b[c * E_LOC : (c + 1) * E_LOC]),
            "w2": np.ascontiguousarray(w2b[c * E_LOC : (c + 1) * E_LOC]),
        }
        for c in range(N_CORES)
    ]


def kernel(x: np.ndarray, w1: np.ndarray, w2: np.ndarray, **_) -> np.ndarray:
    """Full inputs in, full output out; expert-parallel across 8 NeuronCores."""
    nc = _get_program()
    in_maps = _prep_inputs(x, w1, w2)
    res = run_bass_kernel_spmd(nc, in_maps, list(range(N_CORES)))

    full = np.stack(
        [np.asarray(res.results[c]["outT"]) for c in range(N_CORES)], axis=0
    ).astype(np.float32)
    full = full.reshape(E, D4, NTOK)              # [e, d4, tok]
    out = full.transpose(2, 1, 0)                 # [tok, d4, e]
    return np.ascontiguousarray(out.reshape(4, 2048, D4, E), dtype=np.float32)
